# revision 1
# baseline (speedup 1.0000x reference)
"""Masked 3-layer MLP (tanh) on 8 Trainium2 NeuronCores.

Reference computation (B=2048, dims 4096->8192->8192->4096, fp32):
    h1 = tanh(x @ (W1*m1).T + b1)
    h2 = tanh(h1 @ (W2*m2).T + b2)
    out =      h2 @ (W3*m3).T + b3

Strategy: Megatron-style column parallelism on every layer. Core k owns a
1/8 shard of each layer's output features (rows of W). All compute is done
in transposed orientation [features, batch] so that:
  - output features land on PSUM partitions -> per-partition bias + tanh
    fuse into the ScalarE PSUM eviction,
  - each layer's output is exactly the next layer's contraction layout,
    so no transposes are needed anywhere on device.
After layers 1 and 2 an on-chip AllGather concatenates the 8 feature shards
(concatenation is on the leading axis = features). The final layer's shard
outputs are gathered and concatenated on the host.

The mask multiply (W * m) runs on VectorE once per weight element while the
weight panel is DMA'd into SBUF; matmuls run at full rate from the cached
panel.
"""

import os
import sys

import numpy as np

for _p in ("/opt/trn_rl_repo", os.path.expanduser("~/.axon_site/_ro/trn_rl_repo")):
    if os.path.isdir(_p) and _p not in sys.path:
        sys.path.append(_p)

B = 2048
DIMS = [4096, 8192, 8192, 4096]
NCORES = 8
P = 128
FD = 512           # matmul moving free dim == one PSUM bank of fp32
NB = B // FD       # batch blocks
ICK = 4            # K-subtiles (x128 rows) per streamed input chunk
MCK = 4            # K-subtiles per weight/mask load+mask chunk

# Compute dtype: fp16 | bf16 | fp32r | fp32
DTYPE = os.environ.get("BASS_MLP_DTYPE", "fp16")

_cache = {}


def _np_cdt():
    if DTYPE == "bf16":
        import ml_dtypes

        return ml_dtypes.bfloat16
    return {"fp16": np.float16, "fp32r": np.float32, "fp32": np.float32}[DTYPE]


def _build(l1k=DIMS[0]):
    """Build + schedule the SPMD Bass program (same NEFF on all 8 cores).

    l1k: layer-1 contraction size. DIMS[0] for the dense path; a smaller
    multiple of 512 when the host packs only the K-rows that survive m1
    (per-core), padding with zeros.
    """
    import concourse.tile as tile
    from concourse import bacc, mybir
    from concourse.bass import DynSlice

    cdt = {
        "fp16": mybir.dt.float16,
        "bf16": mybir.dt.bfloat16,
        "fp32r": mybir.dt.float32r,  # rounded fp32; np side is float32
        "fp32": mybir.dt.float32,
    }[DTYPE]
    esz = mybir.dt.size(cdt)

    # Per-layer output-feature shard sizes and weight-panel widths.
    FS = [DIMS[1] // NCORES, DIMS[2] // NCORES, DIMS[3] // NCORES]  # 1024,1024,512
    KS = [l1k, DIMS[1], DIMS[2]]
    if esz == 2:
        # Uniform 64KB/partition weight-panel slots so wpool can double-buffer:
        # the next panel's DMA+mask overlaps the current panel's matmuls.
        FBLK = [1024, 512, 512]
        mck, ibufs, wbufs = MCK, 6, 2
    else:
        FBLK = [1024, 512, 512]      # L2 split into two panels (SBUF)
        mck, ibufs, wbufs = 2, 4, 1

    nc = bacc.Bacc(None, target_bir_lowering=False, debug=False, num_devices=NCORES)

    xT = nc.dram_tensor("xT", [KS[0], B], cdt, kind="ExternalInput")
    wts, mts, bs = [], [], []
    for li in range(3):
        wts.append(nc.dram_tensor(f"w{li + 1}t", [KS[li], FS[li]], cdt,
                                  kind="ExternalInput"))
        mts.append(nc.dram_tensor(f"m{li + 1}t", [KS[li], FS[li]], cdt,
                                  kind="ExternalInput"))
        bs.append(nc.dram_tensor(f"b{li + 1}", [FS[li]], mybir.dt.float32,
                                 kind="ExternalInput"))
    out = nc.dram_tensor("out", [FS[2], B], mybir.dt.float32,
                         kind="ExternalOutput")

    with tile.TileContext(nc) as tc:
        with tc.tile_pool(name="wp", bufs=wbufs) as wpool, \
             tc.tile_pool(name="inp", bufs=ibufs) as ipool, \
             tc.tile_pool(name="mp", bufs=2) as mpool, \
             tc.tile_pool(name="op", bufs=6) as opool, \
             tc.tile_pool(name="bp", bufs=3) as bpool, \
             tc.tile_pool(name="ps", bufs=8, space="PSUM") as pspool, \
             tc.tile_pool(name="dram", bufs=1, space="DRAM") as dram:

            # Per-(layer, b-block) activation tensors so each AllGather covers
            # one 512-batch block and pipelines behind compute.
            h_loc = [[dram.tile([FS[li], FD], cdt, name=f"h{li + 1}_loc{b}")
                      for b in range(NB)] for li in range(2)]
            h_full = [[dram.tile([DIMS[li + 1], FD], cdt, addr_space="Shared",
                                 name=f"h{li + 1}_full{b}")
                       for b in range(NB)] for li in range(2)]

            def layer(li, tanh):
                K, F = KS[li], FS[li]
                KO = K // P
                wt_r = wts[li].ap().rearrange("(ko p) f -> p ko f", p=P)
                mt_r = mts[li].ap().rearrange("(ko p) f -> p ko f", p=P)
                if li == 0:
                    xr = xT.ap().rearrange("(ko p) n -> p ko n", p=P)
                    in_rs = [xr[:, :, DynSlice(b * FD, FD)] for b in range(NB)]
                else:
                    in_rs = [h_full[li - 1][b][:].rearrange(
                        "(ko p) n -> p ko n", p=P) for b in range(NB)]

                btile = bpool.tile([P, F // P], mybir.dt.float32, tag="bias",
                                   name=f"bias{li}")
                nc.sync.dma_start(btile[:], bs[li].ap().rearrange(
                    "(o p) -> p o", p=P))

                fblk = FBLK[li]
                for f0 in range(0, F, fblk):
                    # --- load + mask one weight panel [P, KO, fblk] ---
                    wp = wpool.tile([P, KO, fblk], cdt, tag="wpanel",
                                    name=f"wp{li}_{f0}")
                    # weight/mask loads go on gpsimd/vector DMA queues so the
                    # input-strip stream on the sync queue is never stuck
                    # behind a 16MB panel load
                    for c0 in range(0, KO, mck):
                        csl = slice(c0, c0 + mck)
                        fsl = DynSlice(f0, fblk)
                        nc.gpsimd.dma_start(wp[:, csl, :], wt_r[:, csl, fsl])
                        mtile = mpool.tile([P, mck, fblk], cdt, tag="mchunk",
                                           name=f"m{li}_{f0}_{c0}")
                        nc.gpsimd.dma_start(mtile[:], mt_r[:, csl, fsl])
                        nc.vector.tensor_tensor(wp[:, csl, :], wp[:, csl, :],
                                                mtile[:], mybir.AluOpType.mult)

                    nf = fblk // P
                    for b in range(NB):
                        psums = [pspool.tile([P, FD], mybir.dt.float32,
                                             tag="ps", name=f"ps{li}_{f0}_{b}_{f}")
                                 for f in range(nf)]
                        for c0 in range(0, KO, ICK):
                            it = ipool.tile([P, ICK, FD], cdt, tag="instrip",
                                            name=f"in{li}_{f0}_{b}_{c0}")
                            nc.sync.dma_start(
                                it[:], in_rs[b][:, slice(c0, c0 + ICK), :])
                            for f in range(nf):
                                for ks in range(ICK):
                                    ko = c0 + ks
                                    nc.tensor.matmul(
                                        psums[f][:],
                                        wp[:, ko, DynSlice(f * P, P)],
                                        it[:, ks, :],
                                        start=(ko == 0), stop=(ko == KO - 1))
                        for f in range(nf):
                            fg = f0 + f * P   # feature row offset in shard
                            odt = cdt if li < 2 else mybir.dt.float32
                            ot = opool.tile([P, FD], odt, tag="prod",
                                            name=f"o{li}_{f0}_{b}_{f}")
                            func = (mybir.ActivationFunctionType.Tanh if tanh
                                    else mybir.ActivationFunctionType.Identity)
                            nc.scalar.activation(
                                ot[:], psums[f][:], func,
                                bias=btile[:, DynSlice((f0 // P) + f, 1)])
                            if li < 2:
                                nc.sync.dma_start(
                                    h_loc[li][b][DynSlice(fg, P), :], ot[:])
                            else:
                                nc.sync.dma_start(
                                    out.ap()[DynSlice(fg, P),
                                             DynSlice(b * FD, FD)], ot[:])
                        # fire this b-block's AllGather as soon as the last
                        # panel has written it
                        if li < 2 and f0 == F - fblk:
                            nc.gpsimd.collective_compute(
                                "AllGather",
                                mybir.AluOpType.bypass,
                                replica_groups=[list(range(NCORES))],
                                ins=[h_loc[li][b].opt()],
                                outs=[h_full[li][b].opt()],
                            )

            layer(0, tanh=True)
            layer(1, tanh=True)
            layer(2, tanh=False)

    nc.compile()
    return nc


PACK_K = 512   # packed layer-1 contraction size (sparse-mask fast path)


def get_nc(l1k=DIMS[0]):
    if l1k not in _cache:
        _cache[l1k] = _build(l1k)
    return _cache[l1k]


def plan_l1k(m1):
    """If m1 is sparse enough that every core's shard of (W1*m1).T touches at
    most PACK_K input dims, return (PACK_K, per-core used-row indices); else
    the dense plan."""
    m1 = np.asarray(m1)
    fs = DIMS[1] // NCORES
    idxs = []
    for k in range(NCORES):
        idx = np.flatnonzero(m1[k * fs:(k + 1) * fs].any(axis=0))
        if len(idx) > PACK_K:
            return DIMS[0], None
        idxs.append(idx)
    return PACK_K, idxs


def make_in_maps(x, W1, b1, m1, W2, b2, m2, W3, b3, m3, idxs=None):
    """Host-side sharding: transpose to [K, F] layouts, cast, slice shards.
    With idxs, layer-1 operands are gathered to the PACK_K used K-rows."""
    x, W1, b1, m1, W2, b2, m2, W3, b3, m3 = (
        np.asarray(a) for a in (x, W1, b1, m1, W2, b2, m2, W3, b3, m3))
    npdt = _np_cdt()
    xT = np.ascontiguousarray(x.T).astype(npdt, copy=False)
    Ws = [W1, W2, W3]
    Ms = [m1, m2, m3]
    Bs = [b1, b2, b3]
    in_maps = []
    for k in range(NCORES):
        m = {}
        for li in range(3):
            F = DIMS[li + 1]
            fs = F // NCORES
            sl = slice(k * fs, (k + 1) * fs)
            wt = Ws[li][sl].T
            mt = Ms[li][sl].T
            if li == 0:
                if idxs is None:
                    m["xT"] = xT
                else:
                    idx = idxs[k]
                    xk = np.zeros((PACK_K, B), npdt)
                    xk[:len(idx)] = xT[idx]
                    m["xT"] = xk
                    wk = np.zeros((PACK_K, fs), npdt)
                    wk[:len(idx)] = wt[idx].astype(npdt)
                    mk = np.zeros((PACK_K, fs), npdt)
                    mk[:len(idx)] = mt[idx].astype(npdt)
                    m["w1t"], m["m1t"] = wk, mk
            if f"w{li + 1}t" not in m:
                m[f"w{li + 1}t"] = np.ascontiguousarray(wt).astype(
                    npdt, copy=False)
                m[f"m{li + 1}t"] = np.ascontiguousarray(mt).astype(npdt)
            m[f"b{li + 1}"] = np.ascontiguousarray(Bs[li][sl]).astype(
                np.float32, copy=False)
        in_maps.append(m)
    return in_maps


def kernel(x, W1, b1, m1, W2, b2, m2, W3, b3, m3):
    from concourse.bass_utils import run_bass_kernel_spmd

    l1k, idxs = plan_l1k(m1)
    nc = get_nc(l1k)
    in_maps = make_in_maps(x, W1, b1, m1, W2, b2, m2, W3, b3, m3, idxs=idxs)
    res = run_bass_kernel_spmd(nc, in_maps, core_ids=list(range(NCORES)))
    outT = np.concatenate([res.results[k]["out"] for k in range(NCORES)], axis=0)
    return np.ascontiguousarray(outT.T)

